# revision 7
# baseline (speedup 1.0000x reference)
"""Longformer attention Trainium2 kernel (8 NeuronCores, SPMD).

Sharding: data-parallel over batch (cores 0-3 -> batch 0, 4-7 -> batch 1),
head-parallel within a batch group (4 heads = 256 channels per core).
Each core: QKV projection for its head slice, banded+global attention,
out-projection partial; host sums the 4 partials per batch and adds the
bias terms (bo and the fold of bv through Wo).
"""

import numpy as np
import ml_dtypes

import concourse.bacc as bacc
import concourse.mybir as mybir
from concourse.tile import TileContext
from concourse.bass_utils import run_bass_kernel_spmd

S = 2048          # sequence length
D = 1024          # model dim
NH = 16           # total heads
DH = 64           # head dim
HPC = 4           # heads per core
CPB = 4           # cores per batch
WIN = 256         # attention window (2 blocks of 128)
NB = S // 128     # 16 query/key blocks
BF16 = mybir.dt.bfloat16
F32 = mybir.dt.float32

_CACHE = {}


def _band(qb):
    return list(range(max(0, qb - 2), min(NB - 1, qb + 2) + 1))


def _mask_id(qb, kb):
    # 0:M1 lower edge, 1:M1g (+global key row), 2:M2 upper edge, 3:M2g (+global query col)
    if kb == qb - 2:
        return 1 if kb == 0 else 0
    if kb == qb + 2:
        return 3 if qb == 0 else 2
    return None


def build_masks():
    ki = np.arange(128)[:, None]
    qi = np.arange(128)[None, :]
    m1 = (qi <= ki).astype(np.float32)          # kb == qb-2 : valid iff qi <= ki
    m2 = (ki <= qi).astype(np.float32)          # kb == qb+2 : valid iff ki <= qi
    m1g = m1.copy(); m1g[0, :] = 1.0            # global key k=0 row
    m2g = m2.copy(); m2g[:, 0] = 1.0            # global query q=0 col
    return np.stack([m1, m1g, m2, m2g]).astype(ml_dtypes.bfloat16)


def build_program():
    nc = bacc.Bacc("TRN2", target_bir_lowering=False, debug=False, num_devices=8)

    FP8 = mybir.dt.float8e4
    x8d = nc.dram_tensor("x8", [8, 128, S], FP8, kind="ExternalInput").ap()
    dx8d = nc.dram_tensor("dx8", [8, 128, S], FP8, kind="ExternalInput").ap()
    x64d = nc.dram_tensor("x64", [8, 128, S], FP8, kind="ExternalInput").ap()
    wq = nc.dram_tensor("wq", [2, 8, 128, 256], FP8, kind="ExternalInput").ap()
    wk = nc.dram_tensor("wk", [2, 8, 128, 256], FP8, kind="ExternalInput").ap()
    wv = nc.dram_tensor("wv", [2, 8, 128, 256], FP8, kind="ExternalInput").ap()
    wo = nc.dram_tensor("wo", [2, 128, D], BF16, kind="ExternalInput").ap()
    bqd = nc.dram_tensor("bq", [2, 128, 1], F32, kind="ExternalInput").ap()
    bkd = nc.dram_tensor("bk", [2, 128, 1], F32, kind="ExternalInput").ap()
    maskd = nc.dram_tensor("masks", [4, 128, 128], BF16, kind="ExternalInput").ap()
    identd = nc.dram_tensor("ident", [128, 128], BF16, kind="ExternalInput").ap()
    y = nc.dram_tensor("y", [S, D], F32, kind="ExternalOutput").ap()

    with TileContext(nc) as tc:
        import contextlib
        with contextlib.ExitStack() as ctx, \
                nc.allow_low_precision(reason="bf16 attention interior by design"):
            sbw = ctx.enter_context(tc.tile_pool(name="sbw", bufs=1))
            sbx = ctx.enter_context(tc.tile_pool(name="sbx", bufs=1))
            sbqk = ctx.enter_context(tc.tile_pool(name="sbqk", bufs=1))
            sbes = ctx.enter_context(tc.tile_pool(name="sbes", bufs=16))
            sbsm = ctx.enter_context(tc.tile_pool(name="sbsm", bufs=4))
            sbbc = ctx.enter_context(tc.tile_pool(name="sbbc", bufs=4))
            psA = ctx.enter_context(tc.tile_pool(name="psA", bufs=2, space="PSUM"))
            psS = ctx.enter_context(tc.tile_pool(name="psS", bufs=2, space="PSUM"))
            psPV = ctx.enter_context(tc.tile_pool(name="psPV", bufs=2, space="PSUM"))
            
            # ---- load inputs: fp8 x variants (x8, dx8 residual, x8/64)
            # ---- wave by wave so span-0 compute starts early ----
            wqt = sbw.tile([128, 2, 8, 256], FP8, tag="wqt")
            wkt = sbw.tile([128, 2, 8, 256], FP8, tag="wkt")
            wvt = sbw.tile([128, 2, 8, 256], FP8, tag="wvt")
            nc.sync.dma_start(out=wqt[:], in_=wq.rearrange("v e p c -> p v e c"))
            nc.gpsimd.dma_start(out=wkt[:], in_=wk.rearrange("v e p c -> p v e c"))
            xt = [sbx.tile([128, 8, S], FP8, tag=f"x8v{v}", name=f"x8v{v}")
                  for v in range(3)]
            xsrc = (x8d, dx8d, x64d)
            for v in range(3):
                eng = nc.sync if v % 2 else nc.gpsimd
                eng.dma_start(out=xt[v][:, :, 0:512],
                              in_=xsrc[v][:, :, 0:512].rearrange("e p s -> p e s"))
            for v in range(3):
                eng = nc.gpsimd if v % 2 else nc.sync
                eng.dma_start(out=xt[v][:, :, 512:1024],
                              in_=xsrc[v][:, :, 512:1024].rearrange("e p s -> p e s"))
            wot = []
            for cc in range(2):
                t = sbw.tile([128, D], BF16, tag=f"wo{cc}")
                nc.sync.dma_start(out=t[:], in_=wo[cc, :, :])
                wot.append(t)
            bqt, bkt = [], []
            for cc in range(2):
                tq = sbw.tile([128, 1], F32, tag=f"bq{cc}")
                nc.sync.dma_start(out=tq[:], in_=bqd[cc, :, :])
                bqt.append(tq)
                tk = sbw.tile([128, 1], F32, tag=f"bk{cc}")
                nc.sync.dma_start(out=tk[:], in_=bkd[cc, :, :])
                bkt.append(tk)
            ones1 = sbw.tile([1, 128], BF16, tag="ones1")
            nc.vector.memset(ones1[:], 1.0)
            mt = []
            for i in range(4):
                t = sbw.tile([128, 128], BF16, tag=f"mask{i}")
                nc.sync.dma_start(out=t[:], in_=maskd[i, :, :])
                mt.append(t)
            identt = sbw.tile([128, 128], BF16, tag="identt")
            nc.gpsimd.dma_start(out=identt[:], in_=identd)
            nc.sync.dma_start(out=wvt[:], in_=wv.rearrange("v e p c -> p v e c"))
            for v in range(3):
                eng = nc.sync if v % 2 else nc.gpsimd
                eng.dma_start(out=xt[v][:, :, 1024:2048],
                              in_=xsrc[v][:, :, 1024:2048].rearrange("e p s -> p e s"))

            # ---- persistent intermediates ----
            QT = [sbqk.tile([128, S], BF16, tag=f"QT{c}", name=f"QT{c}") for c in range(2)]
            KT = [sbqk.tile([128, S], BF16, tag=f"KT{c}", name=f"KT{c}") for c in range(2)]
            Vo = [sbqk.tile([128, HPC * 65], BF16, tag=f"Vo{t}", name=f"Vo{t}") for t in range(NB)]


            # ---- phase A: projections, emitted span-by-span so the
            # ---- attention pipeline can start after the first spans ----
            DR = mybir.MatmulPerfMode.DoubleRow
            CHAINS = [(0, 0), (1, 0), (2, 1)]   # x8@W8 + dx8@W8 + (x8/64)@(64*dW8)

            def emit_qkt_span(ts):
                sp = slice(ts * 512, (ts + 1) * 512)
                for cc in range(2):
                    cs = slice(cc * 128, (cc + 1) * 128)
                    pq = psA.tile([128, 512], F32, tag="psA", name="pq")
                    for i, (xv, wv_) in enumerate(CHAINS):
                        for j in range(4):
                            nc.tensor.matmul(pq[:], wqt[:, wv_, 2 * j:2 * j + 2, cs],
                                             xt[xv][:, 2 * j:2 * j + 2, sp],
                                             start=(i == 0 and j == 0),
                                             stop=(i == 2 and j == 3), perf_mode=DR)
                    # Q' = (x Wq + bq) / 8 : scale folded in, bias pre-scaled on host
                    nc.vector.tensor_scalar(QT[cc][:, sp], pq[:], 0.125, bqt[cc][:],
                                            mybir.AluOpType.mult, mybir.AluOpType.add)
                    pk = psA.tile([128, 512], F32, tag="psA", name="pk")
                    for i, (xv, wv_) in enumerate(CHAINS):
                        for j in range(4):
                            nc.tensor.matmul(pk[:], wkt[:, wv_, 2 * j:2 * j + 2, cs],
                                             xt[xv][:, 2 * j:2 * j + 2, sp],
                                             start=(i == 0 and j == 0),
                                             stop=(i == 2 and j == 3), perf_mode=DR)
                    nc.vector.tensor_scalar(KT[cc][:, sp], pk[:], bkt[cc][:], None,
                                            mybir.AluOpType.add)
            def emit_v(tb):
                ks = slice(tb * 128, (tb + 1) * 128)
                pv = psA.tile([128, 256], F32, tag="psA", name="pv")
                for i, (xv, wv_) in enumerate(CHAINS):
                    for j in range(4):
                        nc.tensor.matmul(pv[:], xt[xv][:, 2 * j:2 * j + 2, ks],
                                         wvt[:, wv_, 2 * j:2 * j + 2, :],
                                         start=(i == 0 and j == 0),
                                         stop=(i == 2 and j == 3), perf_mode=DR)
                # scatter heads into [h*65 : h*65+64]; col h*65+64 gets ones
                outap = Vo[tb][:, 0:260].rearrange("p (h c) -> p h c", h=4)[:, :, 0:64]
                inap = pv[:].rearrange("p (h c) -> p h c", h=4)
                nc.scalar.activation(outap, inap, mybir.ActivationFunctionType.Copy)
                onesap = Vo[tb][:, 0:260].rearrange("p (h c) -> p h c", h=4)[:, :, 64:65]
                nc.vector.memset(onesap, 1.0)

            # ---- global key (k=0) score rows, batched 4 qb per exp ----
            # esgt[h][g] covers qb 4g..4g+3 as [1, 512]; only slices for qb>=3 used
            esgt = [[None] * 4 for _ in range(HPC)]
            def emit_esg(g):
                for h in range(HPC):
                    hp, r0 = h // 2, (h % 2) * 64
                    psg = psA.tile([128, 512], F32, tag="psA", name="psg")
                    for j in range(4):
                        qb = 4 * g + j
                        if qb < 3:
                            continue
                        nc.tensor.matmul(psg[0:1, j * 128:(j + 1) * 128],
                                         KT[hp][r0:r0 + 64, 0:1],
                                         QT[hp][r0:r0 + 64, qb * 128:(qb + 1) * 128],
                                         start=True, stop=True)
                    eg = sbsm.tile([1, 512], BF16, tag=f"esg{h}_{g}", name="eg")
                    lo = 3 if g == 0 else 0
                    nc.scalar.activation(eg[0:1, lo * 128:512], psg[0:1, lo * 128:512],
                                         mybir.ActivationFunctionType.Exp)
                    esgt[h][g] = eg

            emit_qkt_span(0)
            emit_esg(0)
            emit_qkt_span(1)
            emit_esg(1)

            # ---- banded attention; V tiles emitted just-in-time so exp work
            # ---- starts early; pair 0 (which needs all V for the global row)
            # ---- runs after pair 4
            pair_order = [1, 2, 3, 4, 0, 5, 6, 7]
            v_before = {1: range(0, 6), 2: range(6, 8), 3: range(8, 10),
                        4: range(10, 12), 0: range(12, 16)}
            for pair in pair_order:
                if pair == 2:
                    emit_qkt_span(2)
                    emit_esg(2)
                    emit_qkt_span(3)
                    emit_esg(3)
                for tb in v_before.get(pair, ()):
                    emit_v(tb)
                qb0 = pair * 2
                aots = {}
                for sub in range(2):
                    qb = qb0 + sub
                    qs = slice(qb * 128, (qb + 1) * 128)
                    kbs = _band(qb)
                    w = len(kbs) * 128
                    ess = []
                    for h in range(HPC):
                        hp, r0 = h // 2, (h % 2) * 64
                        pss = psS.tile([128, 1024], F32, tag="psS", name="ps")
                        for i, kb in enumerate(kbs):
                            nc.tensor.matmul(pss[:, i * 128:(i + 1) * 128],
                                             KT[hp][r0:r0 + 64, kb * 128:(kb + 1) * 128],
                                             QT[hp][r0:r0 + 64, qs],
                                             start=True, stop=True)
                        es = sbes.tile([128, 1024], BF16, tag="es", name="es")
                        nc.scalar.activation(es[:, 0:w], pss[:, 0:w],
                                             mybir.ActivationFunctionType.Exp)
                        for i, kb in enumerate(kbs):
                            mid = _mask_id(qb, kb)
                            if mid is not None:
                                sl = slice(i * 128, (i + 1) * 128)
                                nc.vector.tensor_mul(es[:, sl], es[:, sl], mt[mid][:])
                        ess.append(es)
                    # PV with es as the stationary operand: out [q, 4h*65]
                    ppv = psPV.tile([128, 260], F32, tag="ppv", name="ppv")
                    for h in range(HPC):
                        hp, r0 = h // 2, (h % 2) * 64
                        hs = slice(h * 65, h * 65 + 65)
                        es = ess[h]
                        nband = len(kbs) + (1 if qb >= 3 else 0)
                        ij = 0
                        for i, kb in enumerate(kbs):
                            nc.tensor.matmul(ppv[:, hs], es[:, i * 128:(i + 1) * 128],
                                             Vo[kb][:, hs], start=(ij == 0),
                                             stop=(ij == nband - 1))
                            ij += 1
                        if qb >= 3:  # global key k=0
                            eg = esgt[h][qb // 4]
                            co = (qb % 4) * 128
                            nc.tensor.matmul(ppv[:, hs], eg[0:1, co:co + 128],
                                             Vo[0][0:1, hs],
                                             start=False, stop=(ij == nband - 1))
                            ij += 1
                        if qb == 0:  # global query q=0 vs far keys
                            ps0 = psA.tile([128, 512], F32, tag="psA", name="ps0")
                            for i, kb in enumerate(range(3, NB)):
                                nc.tensor.matmul(
                                    ps0[:, i:i + 1],
                                    KT[hp][r0:r0 + 64, kb * 128:(kb + 1) * 128],
                                    QT[hp][r0:r0 + 64, 0:1], start=True, stop=True)
                            es0 = sbsm.tile([128, 13], BF16, tag="es0", name="es0")
                            nc.scalar.activation(es0[:], ps0[:, 0:13],
                                                 mybir.ActivationFunctionType.Exp)
                            pf = psA.tile([1, 260], F32, tag="psA", name="pf")
                            for i, kb in enumerate(range(3, NB)):
                                nc.tensor.matmul(pf[0:1, hs], es0[:, i:i + 1],
                                                 Vo[kb][:, hs],
                                                 start=(i == 0), stop=(i == 12))
                            pfs = sbsm.tile([1, 260], F32, tag="pfs", name="pfs")
                            nc.vector.tensor_copy(pfs[0:1, hs], pf[0:1, hs])
                            nc.vector.tensor_tensor(ppv[0:1, hs], ppv[0:1, hs],
                                                    pfs[0:1, hs],
                                                    mybir.AluOpType.add)
                    # normalize: recip column + per-partition scale, then
                    # transpose [q, c] -> [c, q] via identity matmul
                    rc = sbsm.tile([128, 4], F32, tag="rc", name="rc")
                    ppr = ppv[:].rearrange("p (h c) -> p h c", h=4)
                    nc.vector.reciprocal(rc[:], ppr[:, :, 64:65])
                    ao = sbsm.tile([128, 256], BF16, tag="ao", name="ao")
                    for h in range(HPC):
                        nc.vector.tensor_scalar(ao[:, h * 64:(h + 1) * 64],
                                                ppr[:, h, 0:64], rc[:, h:h + 1],
                                                None, mybir.AluOpType.mult)
                    pt = psA.tile([128, 256], F32, tag="psA", name="pt")
                    for j in range(2):
                        nc.tensor.matmul(pt[:, j * 128:(j + 1) * 128],
                                         ao[:, j * 128:(j + 1) * 128], identt[:],
                                         start=True, stop=True)
                    aot = sbsm.tile([128, 2, 128], BF16, tag="aot", name="aot")
                    nc.vector.tensor_copy(aot[:].rearrange("p a b -> p (a b)"),
                                          pt[:])
                    aots[qb] = aot
                # out projection for this pair from the transposed AO tiles
                for qb2 in (qb0, qb0 + 1):
                    q2 = slice(qb2 * 128, (qb2 + 1) * 128)
                    aot = aots[qb2]
                    for eh in range(2):
                        po = psA.tile([128, 512], F32, tag="psA", name="po")
                        for cc in range(2):
                            nc.tensor.matmul(po[:], aot[:, cc, :],
                                             wot[cc][:, eh * 512:(eh + 1) * 512],
                                             start=(cc == 0), stop=(cc == 1))
                        ys = sbbc.tile([128, 512], F32, tag="ystage", name="ys")
                        if eh == 0:
                            nc.scalar.activation(ys[:], po[:],
                                                 mybir.ActivationFunctionType.Copy)
                        else:
                            nc.vector.tensor_copy(ys[:], po[:])
                        eng = nc.sync if (qb2 + eh) % 2 else nc.gpsimd
                        eng.dma_start(out=y[q2, eh * 512:(eh + 1) * 512], in_=ys[:])

    nc.compile()
    return nc


def kernel(x, Wq, bq, Wk, bk, Wv, bv, Wo, bo):
    x = np.asarray(x); Wq = np.asarray(Wq); bq = np.asarray(bq)
    Wk = np.asarray(Wk); bk = np.asarray(bk); Wv = np.asarray(Wv)
    bv = np.asarray(bv); Wo = np.asarray(Wo); bo = np.asarray(bo)
    if "nc" not in _CACHE:
        _CACHE["nc"] = build_program()
    nc = _CACHE["nc"]

    B = x.shape[0]
    masks = build_masks()
    bf = ml_dtypes.bfloat16
    f8 = ml_dtypes.float8_e4m3

    def wpack(W, sl):
        Wc = np.ascontiguousarray(W[:, sl]).astype(np.float32)
        W8 = Wc.astype(f8)
        dW = ((Wc - W8.astype(np.float32)) * 64.0).astype(f8)
        return np.stack([W8.reshape(8, 128, 256), dW.reshape(8, 128, 256)])

    xTs, dxTs, x64s = [], [], []
    for b in range(B):
        xTf = np.ascontiguousarray(x[b].T).astype(np.float32).reshape(8, 128, S)
        x8 = xTf.astype(f8)
        xTs.append(x8)
        dxTs.append((xTf - x8.astype(np.float32)).astype(f8))
        x64s.append((x8.astype(np.float32) / 64.0).astype(f8))
    in_maps = []
    for c in range(8):
        b = c // CPB
        h0 = (c % CPB) * HPC * DH          # channel offset of this core's heads
        sl = slice(h0, h0 + HPC * DH)
        in_maps.append({
            "x8": xTs[b],
            "dx8": dxTs[b],
            "x64": x64s[b],
            "wq": wpack(Wq, sl),
            "wk": wpack(Wk, sl),
            "wv": wpack(Wv, sl),
            "wo": np.ascontiguousarray(Wo[sl, :]).reshape(2, 128, D).astype(bf),
            "bq": (bq[sl] * 0.125).reshape(2, 128, 1).astype(np.float32),
            "bk": bk[sl].reshape(2, 128, 1).astype(np.float32),
            "masks": masks,
            "ident": np.eye(128, dtype=np.float32).astype(bf),
        })
    res = run_bass_kernel_spmd(nc, in_maps, list(range(8)))
    out = np.zeros((B, S, D), dtype=np.float32)
    for c in range(8):
        out[c // CPB] += res.results[c]["y"]
    out += (bv @ Wo + bo)[None, None, :]
    return out

